# revision 11
# baseline (speedup 1.0000x reference)
"""Trainium2 Bass kernel for nn_DRO_TOPK (margin-loss top-k + masked sim stats).

Strategy (8 NeuronCores, data-parallel over rows, symmetry-halved, maskless):
  - sim = X @ X.T is symmetric: every unordered pair {i, j} is covered once
    by the half-circle band d = (j - i) mod 4096 in [1, 2048]. Each core
    handles 512 rows as 4 row-tiles of 128; per tile it computes a
    [128, 2048] rect (cols [a, a+2048), a = t*128) as two [128, 1024] PSUM
    half-tiles with fp8-e4m3 DoubleRow matmuls (inputs pre-scaled x16, so
    psum holds 256*sim), but only consumes offsets [128, 2048): every cell
    there is a valid, unique pair (d = x - p in [1, 2047]) -- no masks.
  - The two junk/remainder triangles per tile (corner: d in [1, 127-p];
    right: d in [2048-p, 2048]) are tiny 128x128 blocks computed EXACTLY
    on the host in f64 (64 small GEMMs), as are all same-class (positive)
    pairs via class buckets.
  - Tile-framework dependencies are TENSOR-granular, so false sharing is
    broken by construction: X^T streams into 3 separate column-chunk tiles
    (matmuls split at chunk boundaries), each psum half-tile is its own
    tensor (scans of one half never block matmuls of the next), and the
    DVE/ACT outputs live in separate tensors.
  - Device emits, per row and psum half, top-8 of the raw band sims (DVE
    max8 straight from PSUM, chasing the PE stream) and a Sign-accumulator
    zero-loss certificate (no cell <= -0.45*256) on the Scalar engine. PE
    p-state is pre-ramped with warmup matmuls while the first chunk
    streams in; the Sign table is pre-loaded the same way.
  - Host merges: device neg-candidates (fp8-accurate, guarded), exact host
    triangle cells, exact positive-pair losses; margin guards (top-8
    sufficiency, same-class pollution, zero certificates) trigger a full
    numpy fallback if the fast path cannot be proven correct. fp8 value
    noise (<=1e-2) is far inside the measured data margins (>=3e-2) and
    the 2e-2 loss tolerance.
"""

import bisect
import os
import sys

import numpy as np

for _p in ('/opt/trn_rl_repo', '/root/.axon_site/_ro/trn_rl_repo'):
    if os.path.isdir(_p) and _p not in sys.path:
        sys.path.insert(0, _p)

N, D, NCORES = 4096, 512, 8
R = N // NCORES            # 512 rows per core
NT = R // 128              # 4 row-tiles per core
RECT = 2048                # psum rect width per row-tile
LO = 128                   # offsets [LO, RECT) are consumed -> 1920 cells/row
WB = RECT - LO             # 1920
XCOLS = 3 * 128 + RECT     # 2432 cols of rotated X^T each core touches
KK = D // 128              # 4 contraction sub-tiles
CB = [0, 512, 1536, XCOLS]  # X^T chunk-tile bounds
MARGIN, BETA, TOPK = 0.5, 0.0, 20
SCALE = 16.0               # fp8 input scale; psum = SCALE^2 * sim
ZTHR = 0.45                # no-zero certificate: all band sims > -ZTHR
EPS = 1.5e-2               # fp8 matmul error envelope for the guards
NWARM = 4                  # PE p-state warmup matmuls

_prog_cache = {}


def _build_program():
    import concourse.bacc as bacc
    import concourse.mybir as mybir
    from concourse.tile import TileContext

    f32 = mybir.dt.float32
    bf16 = mybir.dt.bfloat16
    fp8 = mybir.dt.float8e4
    Act = mybir.ActivationFunctionType
    DR = mybir.MatmulPerfMode.DoubleRow

    nc = bacc.Bacc('TRN2', target_bir_lowering=False, debug=False)
    xtr_d = nc.dram_tensor('xtr', [128, KK, XCOLS], fp8, kind='ExternalInput')
    outc_d = nc.dram_tensor('outc', [128, 32], f32, kind='ExternalOutput')
    outa_d = nc.dram_tensor('outa', [128, 4], f32, kind='ExternalOutput')

    with TileContext(nc) as tc:
        with (
            tc.tile_pool(name='xts', bufs=1) as xts_pool,
            tc.tile_pool(name='zs', bufs=1) as zs_pool,
            tc.tile_pool(name='small', bufs=1) as small_pool,
            tc.tile_pool(name='ps', bufs=1, space='PSUM') as ps_pool,
        ):
            xc = [xts_pool.tile([128, KK, CB[i + 1] - CB[i]], fp8,
                                tag=f'xc{i}', name=f'xc{i}')
                  for i in range(len(CB) - 1)]
            outc = small_pool.tile([128, 32], f32, tag='outc')
            outa = small_pool.tile([128, 4], f32, tag='outa')
            zdump = zs_pool.tile([128, WB], bf16, tag='zdump')
            zbias = small_pool.tile([128, 1], f32, tag='zbias')
            warm = small_pool.tile([128, 2, 512], fp8, tag='warm')

            # Warmup path: ramp PE p-state + preload the Sign ACT table
            # while the first input chunk is still in flight.
            nc.vector.memset(zbias[:, :], ZTHR * SCALE * SCALE)
            nc.vector.memset(warm[:, :, :], 0.0)
            nc.scalar.activation(zdump[:, 0:1], zbias[:, :], Act.Sign,
                                 bias=zbias[:, :])

            # All input chunks via one DGE: its queues drain FIFO, so chunk 0
            # finishes first instead of sharing bandwidth with chunks 1-2.
            nc.gpsimd.dma_start(xc[0][:, :, :], xtr_d[:, :, CB[0]:CB[1]])
            nc.gpsimd.dma_start(xc[1][:, :, :], xtr_d[:, :, CB[1]:CB[2]])
            nc.gpsimd.dma_start(xc[2][:, :, :], xtr_d[:, :, CB[2]:CB[3]])

            # Two explicit [128, 2048] psum buffers (4 banks each), row-tile
            # t -> buffer t%2: scans of tile t never share a tensor with the
            # matmuls of tile t+1, so nothing falsely serializes.
            ps_bufs = [ps_pool.tile([128, RECT], f32, tag=f'psb{j}',
                                    name=f'psb{j}') for j in range(2)]
            for _ in range(NWARM):
                nc.tensor.matmul(ps_bufs[0][:, 0:512], warm[:, :, 0:128],
                                 warm[:, :, 0:512], start=True, stop=True,
                                 perf_mode=DR)

            for t in range(NT):
                a = t * 128
                ps = ps_bufs[t % 2]
                for h in range(4):
                    glo = a + h * 512
                    # split the 512-col group at X^T chunk boundaries
                    cuts = ([glo] + [b for b in CB[1:-1]
                                     if glo < b < glo + 512] + [glo + 512])
                    for plo, phi in zip(cuts, cuts[1:]):
                        ci = bisect.bisect_right(CB, plo) - 1
                        for kk2 in (0, 2):
                            nc.tensor.matmul(
                                ps[:, plo - a:phi - a],
                                xc[0][:, kk2:kk2 + 2, a:a + 128],
                                xc[ci][:, kk2:kk2 + 2,
                                       plo - CB[ci]:phi - CB[ci]],
                                start=(kk2 == 0), stop=(kk2 == 2),
                                perf_mode=DR)
                nc.vector.max(outc[:, t * 8:(t + 1) * 8], ps[:, LO:RECT])
                nc.scalar.activation(zdump[:, :], ps[:, LO:RECT], Act.Sign,
                                     bias=zbias[:, :],
                                     accum_out=outa[:, t:t + 1])

            nc.gpsimd.dma_start(outc_d[:, :], outc[:, :])
            nc.scalar.dma_start(outa_d[:, :], outa[:, :])

    nc.compile()
    return nc


def _numpy_fallback(x, t):
    """Faithful f32 numpy recompute of the full reference (safety net)."""
    sim = x @ x.T
    same = t[:, None] == t[None, :]
    eye = np.eye(N, dtype=bool)
    pos = same & ~eye
    neg = ~same
    pos_l = np.maximum(MARGIN + BETA - sim, 0.0).astype(np.float32)
    neg_l = np.maximum(MARGIN + sim - BETA, 0.0).astype(np.float32)
    valid = pos | neg
    pair = np.where(pos, pos_l, neg_l)
    zeros = int((valid & (pair == 0.0)).sum())
    masked = np.where(valid, pair, -np.inf).ravel()
    top = np.sort(masked)[-TOPK:]
    loss = np.float32(top.astype(np.float64).mean())
    mean_pos = np.float32(sim[pos].astype(np.float64).sum() / pos.sum())
    mean_neg = np.float32(sim[neg].astype(np.float64).sum() / neg.sum())
    return loss, np.int32(zeros), mean_pos, mean_neg


def kernel(**inputs):
    import ml_dtypes
    from concourse.bass_utils import run_bass_kernel_spmd

    x = np.ascontiguousarray(inputs['inputs'].astype(np.float32, copy=False))
    t = np.asarray(inputs['targets'])
    t_i = t.astype(np.int64)

    if 'nc' not in _prog_cache:
        _prog_cache['nc'] = _build_program()
    nc = _prog_cache['nc']

    xq = (x * SCALE).astype(ml_dtypes.float8_e4m3)      # RNE quantization
    xt = np.ascontiguousarray(xq.T)                     # [D, N] fp8
    xt2 = np.concatenate([xt, xt[:, :XCOLS]], axis=1)   # wrap for rotation
    in_maps = []
    for c in range(NCORES):
        sh = c * R
        in_maps.append({
            'xtr': np.ascontiguousarray(
                xt2[:, sh:sh + XCOLS].reshape(KK, 128, XCOLS)
                .transpose(1, 0, 2)),
        })

    res = run_bass_kernel_spmd(nc, in_maps, core_ids=list(range(NCORES)))

    inv = 1.0 / (SCALE * SCALE)
    cands, accs = [], []
    for r in res.results:
        oc = r['outc']                                  # [128, 32]
        oa = r['outa']                                  # [128, 4]
        # cand[t*128+p, j] = oc[p, t*8 + j]
        cands.append(oc.reshape(128, NT, 8).transpose(1, 0, 2).reshape(R, 8))
        accs.append(oa.T.reshape(R))
    cand = np.concatenate(cands, axis=0) * inv          # [N, 8] band sims
    acc = np.concatenate(accs, axis=0)                  # [N] sign accums

    x64 = x.astype(np.float64)

    # ---- exact host triangles: 32 corner + 32 right [128,128] blocks ----
    Xb = x64.reshape(32, 128, D)
    Xs = np.roll(x64, -RECT, axis=0).reshape(32, 128, D)
    CA = Xb @ Xb.transpose(0, 2, 1)                     # corner blocks
    RB = Xb @ Xs.transpose(0, 2, 1)                     # right blocks
    tb = t_i.reshape(32, 128)
    ts = np.roll(t_i, -RECT).reshape(32, 128)
    iu0, iu1 = np.triu_indices(128, 1)
    il0, il1 = np.tril_indices(128, -1)
    corner_s = CA[:, iu0, iu1].ravel()
    corner_same = (tb[:, iu0] == tb[:, iu1]).ravel()
    right_s = RB[:, il0, il1].ravel()
    right_same = (tb[:, il0] == ts[:, il1]).ravel()
    anti_s = RB[:16].diagonal(axis1=1, axis2=2).ravel()
    anti_same = (tb[:16] == ts[:16]).ravel()
    host_neg = np.concatenate([corner_s[~corner_same], right_s[~right_same],
                               anti_s[~anti_same]])
    host_cells = np.concatenate([corner_s, right_s, anti_s])

    # ---- all same-class (positive) pairs exactly, via class buckets ----
    order = np.argsort(t_i, kind='stable')
    ts_sorted = t_i[order]
    starts = np.flatnonzero(np.r_[True, ts_sorted[1:] != ts_sorted[:-1]])
    ends = np.r_[starts[1:], N]
    pos_sims = []
    for s0, s1 in zip(starts, ends):
        if s1 - s0 < 2:
            continue
        idx = order[s0:s1]
        S = x64[idx] @ x64[idx].T
        pos_sims.append(S[np.triu_indices(s1 - s0, 1)])
    pos_sims = (np.concatenate(pos_sims) if pos_sims
                else np.empty(0, np.float64))
    max_same = pos_sims.max() if pos_sims.size else -np.inf

    # ---- merge candidate losses, take top-10 unique pairs ----
    merged = np.concatenate([MARGIN + cand.ravel(),     # device neg cands
                             MARGIN + host_neg,         # exact host neg cells
                             MARGIN - pos_sims])        # exact pos pairs
    top10 = np.sort(merged)[-(TOPK // 2):]
    T = top10[0]

    # ---- guards: prove the fast path exact, else fall back ----
    g8 = cand[:, 7]                                     # per-row 8th largest
    ok = (
        bool(np.all(acc == float(WB)))                  # no cell <= -0.45
        and MARGIN + g8.max() + EPS < T                 # top-8 sufficiency
        and MARGIN + max_same + EPS < T                 # no same-class leak
        and host_cells.min() > -ZTHR                    # host cells zero-free
        and (not pos_sims.size or max_same < ZTHR)
        and T > MARGIN + 0.05                           # sane top values
    )
    if not ok:
        return _numpy_fallback(x, t_i)

    loss = np.float32(top10.mean())
    num_zeros = 0

    # ---- exact f64 stats on host ----
    G = np.zeros((int(t_i.max()) + 1, D), dtype=np.float64)
    np.add.at(G, t_i, x64)
    cls_sq = float((G * G).sum())
    diag_sq = float((x64 * x64).sum())
    cnt = np.bincount(t_i)
    pos_cnt = int((cnt.astype(np.int64) * (cnt - 1)).sum())
    neg_cnt = N * N - int((cnt.astype(np.int64) ** 2).sum())
    tot = x64.sum(axis=0)
    total_sq = float(tot @ tot)
    mean_pos = np.float32((cls_sq - diag_sq) / pos_cnt)
    mean_neg = np.float32((total_sq - cls_sq) / neg_cnt)

    return loss, np.int32(num_zeros), mean_pos, mean_neg


# revision 14
# speedup vs baseline: 1.2262x; 1.2262x over previous
"""Trainium2 Bass kernel for nn_DRO_TOPK (margin-loss top-k + masked sim stats).

Strategy (8 NeuronCores, data-parallel over rows, symmetry-halved, maskless):
  - sim = X @ X.T is symmetric: every unordered pair {i, j} is covered once
    by the half-circle band d = (j - i) mod 4096 in [1, 2048]. Each core
    handles 512 rows as 4 row-tiles of 128; per tile it computes a
    [128, 2048] rect (cols [a, a+2048), a = t*128) as two [128, 1024] PSUM
    half-tiles with fp8-e4m3 DoubleRow matmuls (inputs pre-scaled x16, so
    psum holds 256*sim), but only consumes offsets [128, 2048): every cell
    there is a valid, unique pair (d = x - p in [1, 2047]) -- no masks.
  - The two junk/remainder triangles per tile (corner: d in [1, 127-p];
    right: d in [2048-p, 2048]) are tiny 128x128 blocks computed EXACTLY
    on the host in f64 (64 small GEMMs), as are all same-class (positive)
    pairs via class buckets.
  - Tile-framework dependencies are TENSOR-granular, so false sharing is
    broken by construction: X^T streams into 3 separate column-chunk tiles
    (matmuls split at chunk boundaries), each psum half-tile is its own
    tensor (scans of one half never block matmuls of the next), and the
    DVE/ACT outputs live in separate tensors.
  - Device emits, per row and psum half, top-8 of the raw band sims (DVE
    max8 straight from PSUM, chasing the PE stream) and a Sign-accumulator
    zero-loss certificate (no cell <= -0.45*256) on the Scalar engine. PE
    p-state is pre-ramped with warmup matmuls while the first chunk
    streams in; the Sign table is pre-loaded the same way.
  - Host merges: device neg-candidates (fp8-accurate, guarded), exact host
    triangle cells, exact positive-pair losses; margin guards (top-8
    sufficiency, same-class pollution, zero certificates) trigger a full
    numpy fallback if the fast path cannot be proven correct. fp8 value
    noise (<=1e-2) is far inside the measured data margins (>=3e-2) and
    the 2e-2 loss tolerance.
"""

import bisect
import os
import sys

import numpy as np

for _p in ('/opt/trn_rl_repo', '/root/.axon_site/_ro/trn_rl_repo'):
    if os.path.isdir(_p) and _p not in sys.path:
        sys.path.insert(0, _p)

N, D, NCORES = 4096, 512, 8
R = N // NCORES            # 512 rows per core
NT = R // 128              # 4 row-tiles per core
RECT = 2048                # psum rect width per row-tile
LO = 128                   # offsets [LO, RECT) are consumed -> 1920 cells/row
WB = RECT - LO             # 1920
XCOLS = 3 * 128 + RECT     # 2432 cols of rotated X^T each core touches
KK = D // 128              # 4 contraction sub-tiles
CB = [0, 512, 1536, XCOLS]  # X^T chunk-tile bounds
MARGIN, BETA, TOPK = 0.5, 0.0, 20
SCALE = 16.0               # fp8 input scale; psum = SCALE^2 * sim
ZTHR = 0.45                # no-zero certificate: all band sims > -ZTHR
EPS = 1.5e-2               # fp8 matmul error envelope for the guards
NWARM = 4                  # PE p-state warmup matmuls

_prog_cache = {}


def _build_program():
    import concourse.bacc as bacc
    import concourse.mybir as mybir
    from concourse.tile import TileContext

    f32 = mybir.dt.float32
    bf16 = mybir.dt.bfloat16
    fp8 = mybir.dt.float8e4
    Act = mybir.ActivationFunctionType
    DR = mybir.MatmulPerfMode.DoubleRow

    nc = bacc.Bacc('TRN2', target_bir_lowering=False, debug=False)
    xtr_d = nc.dram_tensor('xtr', [128, KK, XCOLS], fp8, kind='ExternalInput')
    outc_d = nc.dram_tensor('outc', [128, 64], f32, kind='ExternalOutput')
    outa_d = nc.dram_tensor('outa', [128, 8], f32, kind='ExternalOutput')

    with TileContext(nc) as tc:
        with (
            tc.tile_pool(name='xts', bufs=1) as xts_pool,
            tc.tile_pool(name='zs', bufs=1) as zs_pool,
            tc.tile_pool(name='small', bufs=1) as small_pool,
            tc.tile_pool(name='ps', bufs=1, space='PSUM') as ps_pool,
        ):
            xc = [xts_pool.tile([128, KK, CB[i + 1] - CB[i]], fp8,
                                tag=f'xc{i}', name=f'xc{i}')
                  for i in range(len(CB) - 1)]
            outc = small_pool.tile([128, 64], f32, tag='outc')
            outa = small_pool.tile([128, 8], f32, tag='outa')
            zdump = zs_pool.tile([128, 1024], bf16, tag='zdump')
            zbias = small_pool.tile([128, 1], f32, tag='zbias')
            warm = small_pool.tile([128, 2, 512], fp8, tag='warm')

            # Warmup path: ramp PE p-state + preload the Sign ACT table
            # while the first input chunk is still in flight.
            nc.vector.memset(zbias[:, :], ZTHR * SCALE * SCALE)
            nc.vector.memset(warm[:, :, :], 0.0)
            nc.scalar.activation(zdump[:, 0:1], zbias[:, :], Act.Sign,
                                 bias=zbias[:, :])

            # Chunk 0 via the (idle) sync DGE; later chunks via gpsimd whose
            # ~1.1us/config serialization naturally staggers their transfers
            # behind chunk 0 instead of stealing its bandwidth.
            nc.sync.dma_start(xc[0][:, :, :], xtr_d[:, :, CB[0]:CB[1]])
            nc.gpsimd.dma_start(xc[1][:, :, :], xtr_d[:, :, CB[1]:CB[2]])
            nc.gpsimd.dma_start(xc[2][:, :, :], xtr_d[:, :, CB[2]:CB[3]])

            # Four explicit [128, 1024] psum half-tile buffers (2 banks
            # each); half-slot (t, hh) -> buffer (2t+hh) % 4. Scans of one
            # half never share a tensor with later matmuls until 4 halves
            # later, so nothing falsely serializes.
            ps_bufs = [ps_pool.tile([128, 1024], f32, tag=f'psb{j}',
                                    name=f'psb{j}') for j in range(4)]
            for _ in range(NWARM):
                nc.tensor.matmul(ps_bufs[0][:, 0:512], warm[:, :, 0:128],
                                 warm[:, :, 0:512], start=True, stop=True,
                                 perf_mode=DR)

            for t in range(NT):
                a = t * 128
                for hh in range(2):
                    ps = ps_bufs[(2 * t + hh) % 4]
                    base = a + hh * 1024
                    for h2 in range(2):
                        glo = base + h2 * 512
                        # split the 512-col group at X^T chunk boundaries
                        cuts = ([glo] + [b for b in CB[1:-1]
                                         if glo < b < glo + 512] + [glo + 512])
                        for plo, phi in zip(cuts, cuts[1:]):
                            ci = bisect.bisect_right(CB, plo) - 1
                            for kk2 in (0, 2):
                                nc.tensor.matmul(
                                    ps[:, plo - base:phi - base],
                                    xc[0][:, kk2:kk2 + 2, a:a + 128],
                                    xc[ci][:, kk2:kk2 + 2,
                                           plo - CB[ci]:phi - CB[ci]],
                                    start=(kk2 == 0), stop=(kk2 == 2),
                                    perf_mode=DR)
                    slot = t * 2 + hh
                    s0 = LO if hh == 0 else 0
                    nc.vector.max(outc[:, slot * 8:(slot + 1) * 8],
                                  ps[:, s0:1024])
                    nc.scalar.activation(zdump[:, 0:1024 - s0],
                                         ps[:, s0:1024], Act.Sign,
                                         bias=zbias[:, :],
                                         accum_out=outa[:, slot:slot + 1])

            nc.gpsimd.dma_start(outc_d[:, :], outc[:, :])
            nc.scalar.dma_start(outa_d[:, :], outa[:, :])

    nc.compile()
    return nc


def _numpy_fallback(x, t):
    """Faithful f32 numpy recompute of the full reference (safety net)."""
    sim = x @ x.T
    same = t[:, None] == t[None, :]
    eye = np.eye(N, dtype=bool)
    pos = same & ~eye
    neg = ~same
    pos_l = np.maximum(MARGIN + BETA - sim, 0.0).astype(np.float32)
    neg_l = np.maximum(MARGIN + sim - BETA, 0.0).astype(np.float32)
    valid = pos | neg
    pair = np.where(pos, pos_l, neg_l)
    zeros = int((valid & (pair == 0.0)).sum())
    masked = np.where(valid, pair, -np.inf).ravel()
    top = np.sort(masked)[-TOPK:]
    loss = np.float32(top.astype(np.float64).mean())
    mean_pos = np.float32(sim[pos].astype(np.float64).sum() / pos.sum())
    mean_neg = np.float32(sim[neg].astype(np.float64).sum() / neg.sum())
    return loss, np.int32(zeros), mean_pos, mean_neg


def kernel(**inputs):
    import ml_dtypes
    from concourse.bass_utils import run_bass_kernel_spmd

    x = np.ascontiguousarray(inputs['inputs'].astype(np.float32, copy=False))
    t = np.asarray(inputs['targets'])
    t_i = t.astype(np.int64)

    if 'nc' not in _prog_cache:
        _prog_cache['nc'] = _build_program()
    nc = _prog_cache['nc']

    xq = (x * SCALE).astype(ml_dtypes.float8_e4m3)      # RNE quantization
    xt = np.ascontiguousarray(xq.T)                     # [D, N] fp8
    xt2 = np.concatenate([xt, xt[:, :XCOLS]], axis=1)   # wrap for rotation
    in_maps = []
    for c in range(NCORES):
        sh = c * R
        in_maps.append({
            'xtr': np.ascontiguousarray(
                xt2[:, sh:sh + XCOLS].reshape(KK, 128, XCOLS)
                .transpose(1, 0, 2)),
        })

    res = run_bass_kernel_spmd(nc, in_maps, core_ids=list(range(NCORES)))

    inv = 1.0 / (SCALE * SCALE)
    cands, accs = [], []
    for r in res.results:
        oc = r['outc']                                  # [128, 64]
        oa = r['outa']                                  # [128, 8]
        # cand[t*128+p, hh*8+j] = oc[p, (t*2+hh)*8 + j]
        cands.append(oc.reshape(128, NT, 16).transpose(1, 0, 2)
                     .reshape(R, 16))
        accs.append(oa.reshape(128, NT, 2).transpose(1, 0, 2).reshape(R, 2))
    cand = np.concatenate(cands, axis=0) * inv          # [N, 16] band sims
    acc = np.concatenate(accs, axis=0)                  # [N, 2] sign accums

    x64 = x.astype(np.float64)

    # ---- exact host triangles: 32 corner + 32 right [128,128] blocks ----
    Xb = x64.reshape(32, 128, D)
    Xs = np.roll(x64, -RECT, axis=0).reshape(32, 128, D)
    CA = Xb @ Xb.transpose(0, 2, 1)                     # corner blocks
    RB = Xb @ Xs.transpose(0, 2, 1)                     # right blocks
    tb = t_i.reshape(32, 128)
    ts = np.roll(t_i, -RECT).reshape(32, 128)
    iu0, iu1 = np.triu_indices(128, 1)
    il0, il1 = np.tril_indices(128, -1)
    corner_s = CA[:, iu0, iu1].ravel()
    corner_same = (tb[:, iu0] == tb[:, iu1]).ravel()
    right_s = RB[:, il0, il1].ravel()
    right_same = (tb[:, il0] == ts[:, il1]).ravel()
    anti_s = RB[:16].diagonal(axis1=1, axis2=2).ravel()
    anti_same = (tb[:16] == ts[:16]).ravel()
    host_neg = np.concatenate([corner_s[~corner_same], right_s[~right_same],
                               anti_s[~anti_same]])
    host_cells = np.concatenate([corner_s, right_s, anti_s])

    # ---- all same-class (positive) pairs exactly, via class buckets ----
    order = np.argsort(t_i, kind='stable')
    ts_sorted = t_i[order]
    starts = np.flatnonzero(np.r_[True, ts_sorted[1:] != ts_sorted[:-1]])
    ends = np.r_[starts[1:], N]
    pos_sims = []
    for s0, s1 in zip(starts, ends):
        if s1 - s0 < 2:
            continue
        idx = order[s0:s1]
        S = x64[idx] @ x64[idx].T
        pos_sims.append(S[np.triu_indices(s1 - s0, 1)])
    pos_sims = (np.concatenate(pos_sims) if pos_sims
                else np.empty(0, np.float64))
    max_same = pos_sims.max() if pos_sims.size else -np.inf

    # ---- merge candidate losses, take top-10 unique pairs ----
    merged = np.concatenate([MARGIN + cand.ravel(),     # device neg cands
                             MARGIN + host_neg,         # exact host neg cells
                             MARGIN - pos_sims])        # exact pos pairs
    top10 = np.sort(merged)[-(TOPK // 2):]
    T = top10[0]

    # ---- guards: prove the fast path exact, else fall back ----
    g8 = cand.reshape(N, 2, 8)[:, :, 7]                 # per-(row,half) 8th
    ok = (
        bool(np.all(acc == np.array([896.0, 1024.0])))  # no cell <= -0.45
        and MARGIN + g8.max() + EPS < T                 # top-8 sufficiency
        and MARGIN + max_same + EPS < T                 # no same-class leak
        and host_cells.min() > -ZTHR                    # host cells zero-free
        and (not pos_sims.size or max_same < ZTHR)
        and T > MARGIN + 0.05                           # sane top values
    )
    if not ok:
        return _numpy_fallback(x, t_i)

    loss = np.float32(top10.mean())
    num_zeros = 0

    # ---- exact f64 stats on host ----
    G = np.zeros((int(t_i.max()) + 1, D), dtype=np.float64)
    np.add.at(G, t_i, x64)
    cls_sq = float((G * G).sum())
    diag_sq = float((x64 * x64).sum())
    cnt = np.bincount(t_i)
    pos_cnt = int((cnt.astype(np.int64) * (cnt - 1)).sum())
    neg_cnt = N * N - int((cnt.astype(np.int64) ** 2).sum())
    tot = x64.sum(axis=0)
    total_sq = float(tot @ tot)
    mean_pos = np.float32((cls_sq - diag_sq) / pos_cnt)
    mean_neg = np.float32((total_sq - cls_sq) / neg_cnt)

    return loss, np.int32(num_zeros), mean_pos, mean_neg
